# revision 4
# baseline (speedup 1.0000x reference)
"""Trainium2 Bass kernel for nn_MCPBRNN_Generic_PETconstraint_constantoutput_
variableLoss_BYPASSM0 (scalar-state nonlinear recurrence over T=50000).

Strategy: Picard fixed-point iteration on the whole trajectory.
  c[t+1] = f(c[t], u[t]) * c[t] + u1[t]  with f nonlinear in c.
Each sweep computes F[t] elementwise from the current trajectory guess, then
re-scans with the DVE's native tensor_tensor_scan (linear recurrence in one
instruction). Strong per-step contraction (f<1-oo, 27% hard resets) gives
convergence to the exact fp32 fixed point in ~20 sweeps.

Layout: 8 cores x 128 partition-chunks of L=50 steps (core k owns
t in [k*6250, (k+1)*6250)). Partition 0 is a warmup chunk (state 0 ->
contraction makes the p=1 boundary exact); chunk boundaries are passed
between partitions each sweep via a PE permutation matmul.
"""
import numpy as np

T = 50000
SPIN_LEN = 365
TRAIN_LEN = 40000
ML = 2.9086
SL = 1.898
NCORES = 8
B = T // NCORES           # 6250
L = 50
NREAL = B // L            # 125
NSWEEP = 22
TINY = 1e-30
YO_COLS = 310             # 128*310 >= 39635
f32 = np.float32

_RUNNER = None


def _build():
    from concourse import bacc, bass, tile, mybir
    dt = mybir.dt.float32
    Alu = mybir.AluOpType
    Act = mybir.ActivationFunctionType

    nc = bacc.Bacc("TRN2", target_bir_lowering=False, debug=False,
                   num_devices=NCORES)
    u1d = nc.dram_tensor("u1", [128, L], dt, kind="ExternalInput").ap()
    u2d = nc.dram_tensor("u2", [128, L], dt, kind="ExternalInput").ap()
    wd = nc.dram_tensor("w", [128, 8], dt, kind="ExternalInput").ap()
    yod = nc.dram_tensor("yo", [128, YO_COLS], dt, kind="ExternalInput").ap()
    permd = nc.dram_tensor("perm", [128, 128], dt, kind="ExternalInput").ap()
    o_c = nc.dram_tensor("o_c", [128, L], dt, kind="ExternalOutput").ap()
    o_f = nc.dram_tensor("o_f", [128, L], dt, kind="ExternalOutput").ap()
    o_olc = nc.dram_tensor("o_olc", [128, L], dt, kind="ExternalOutput").ap()
    o_ol = nc.dram_tensor("o_ol", [128, L], dt, kind="ExternalOutput").ap()
    o_h = nc.dram_tensor("o_h", [128, L], dt, kind="ExternalOutput").ap()
    o_l = nc.dram_tensor("o_l", [128, L], dt, kind="ExternalOutput").ap()
    o_lc = nc.dram_tensor("o_lc", [128, L], dt, kind="ExternalOutput").ap()
    o_std = nc.dram_tensor("o_std", [1, 1], dt, kind="ExternalOutput").ap()

    n = TRAIN_LEN - SPIN_LEN  # 39635

    with tile.TileContext(nc) as tc:
        with (
            tc.tile_pool(name="sb", bufs=1) as pool,
            tc.tile_pool(name="ps", bufs=1, space=bass.MemorySpace.PSUM) as psum,
        ):
            U1 = pool.tile([128, L], dt, tag="U1")
            U2 = pool.tile([128, L], dt, tag="U2")
            W = pool.tile([128, 8], dt, tag="W")
            YO = pool.tile([128, YO_COLS], dt, tag="YO")
            PERM = pool.tile([128, 128], dt, tag="PERM")
            nc.sync.dma_start(U1[:, :], u1d)
            nc.sync.dma_start(U2[:, :], u2d)
            nc.sync.dma_start(W[:, :], wd)
            nc.sync.dma_start(YO[:, :], yod)
            nc.sync.dma_start(PERM[:, :], permd)

            # --- weight prep ---
            expw = pool.tile([128, 3], dt, tag="expw")
            nc.scalar.activation(expw[:, :], W[:, 0:3], Act.Exp)
            den = pool.tile([128, 1], dt, tag="den")
            nc.vector.tensor_reduce(den[:, :], expw[:, :], mybir.AxisListType.X,
                                    Alu.add)
            rden = pool.tile([128, 1], dt, tag="rden")
            nc.vector.reciprocal(rden[:, :], den[:, :])
            oo = pool.tile([128, 1], dt, tag="oo")
            nc.vector.tensor_tensor(oo[:, :], expw[:, 0:1], rden[:, :], Alu.mult)
            ol1a = pool.tile([128, 1], dt, tag="ol1a")
            nc.vector.tensor_tensor(ol1a[:, :], expw[:, 1:2], rden[:, :], Alu.mult)
            scale = pool.tile([128, 1], dt, tag="scale")
            nc.vector.tensor_scalar(scale[:, :], W[:, 4:5], float(1.0 / SL), None,
                                    Alu.mult)
            biasap = pool.tile([128, 1], dt, tag="biasap")
            nc.vector.scalar_tensor_tensor(biasap[:, :], W[:, 4:5],
                                           float(-ML / SL), W[:, 3:4],
                                           Alu.mult, Alu.add)

            # --- per-step gate prefactors ---
            OL = pool.tile([128, L], dt, tag="OL")
            nc.scalar.activation(OL[:, :], U2[:, :], Act.Sigmoid,
                                 bias=biasap[:, :], scale=scale[:, :])
            nc.vector.tensor_scalar(OL[:, :], OL[:, :], ol1a[:, :], None,
                                    Alu.mult)
            EOL = pool.tile([128, L], dt, tag="EOL")
            nc.scalar.activation(EOL[:, :], OL[:, :], Act.Exp)

            # --- Picard sweeps ---
            CT = pool.tile([128, L + 1], dt, tag="CT")
            nc.vector.memset(CT[:, :], TINY)
            rC = pool.tile([128, L], dt, tag="rC")
            pp = pool.tile([128, L], dt, tag="pp")
            Xm = pool.tile([128, L], dt, tag="Xm")
            m1 = pool.tile([128, L], dt, tag="m1")
            X = pool.tile([128, L], dt, tag="X")
            s1 = pool.tile([128, L], dt, tag="s1")
            F = pool.tile([128, L], dt, tag="F")
            bps = psum.tile([128, 1], dt, tag="bps")
            TINYS = pool.tile([128, 1], dt, tag="TINYS")
            nc.vector.memset(TINYS[:, :], TINY)

            def elementwise():
                nc.vector.reciprocal(rC[:, :], CT[:, 0:L])
                nc.vector.tensor_tensor(pp[:, :], U2[:, :], rC[:, :], Alu.mult)
                nc.scalar.activation(Xm[:, :], pp[:, :], Act.Exp, scale=-1.0)
                nc.vector.tensor_tensor(m1[:, :], pp[:, :], OL[:, :], Alu.min)
                nc.vector.tensor_tensor(X[:, :], EOL[:, :], Xm[:, :], Alu.mult)
                nc.vector.scalar_tensor_tensor(s1[:, :], X[:, :], 1.0, m1[:, :],
                                               Alu.min, Alu.subtract)
                nc.vector.tensor_scalar(F[:, :], s1[:, :], oo[:, :], 0.0,
                                        Alu.subtract, Alu.max)

            for sweep in range(NSWEEP):
                elementwise()
                nc.vector.tensor_tensor_scan(CT[:, 1:L + 1], F[:, :], U1[:, :],
                                             CT[:, 0:1], Alu.mult, Alu.add)
                nc.tensor.matmul(bps[:, :], PERM[:, :], CT[:, L:L + 1])
                nc.vector.tensor_tensor(CT[:, 0:1], bps[:, :], TINYS[:, :],
                                        Alu.add)

            # --- final elementwise + outputs ---
            elementwise()
            E = pool.tile([128, L], dt, tag="E")
            nc.vector.tensor_scalar(E[:, :], X[:, :], 1.0, None, Alu.min)
            OLC = pool.tile([128, L], dt, tag="OLC")
            nc.vector.scalar_tensor_tensor(OLC[:, :], m1[:, :], 1.0, E[:, :],
                                           Alu.add, Alu.subtract)
            Hn = pool.tile([128, L], dt, tag="Hn")
            nc.vector.tensor_scalar(Hn[:, :], CT[:, 0:L], oo[:, :], None,
                                    Alu.mult)
            Ln = pool.tile([128, L], dt, tag="Ln")
            nc.vector.tensor_tensor(Ln[:, :], OL[:, :], CT[:, 0:L], Alu.mult)
            LCn = pool.tile([128, L], dt, tag="LCn")
            nc.vector.tensor_tensor(LCn[:, :], OLC[:, :], CT[:, 0:L], Alu.mult)

            # --- obs_std ---
            SQ = pool.tile([128, YO_COLS], dt, tag="SQ")
            s2p = pool.tile([128, 1], dt, tag="s2p")
            nc.scalar.activation(SQ[:, :], YO[:, :], Act.Square,
                                 accum_out=s2p[:, :])
            RS = pool.tile([128, 2], dt, tag="RS")
            nc.vector.tensor_reduce(RS[:, 0:1], YO[:, :], mybir.AxisListType.X,
                                    Alu.add)
            nc.vector.tensor_scalar(RS[:, 1:2], s2p[:, :], 1.0, None, Alu.mult)
            ONES = pool.tile([128, 1], dt, tag="ONES")
            nc.vector.memset(ONES[:, :], 1.0)
            rps = psum.tile([1, 2], dt, tag="rps")
            nc.tensor.matmul(rps[:, :], ONES[:, :], RS[:, :])
            a1 = pool.tile([1, 1], dt, tag="a1")
            nc.scalar.activation(a1[:, :], rps[0:1, 0:1], Act.Square)
            b1 = pool.tile([1, 1], dt, tag="b1")
            nc.vector.tensor_scalar(b1[:, :], a1[:, :], float(1.0 / n), None,
                                    Alu.mult)
            c1 = pool.tile([1, 1], dt, tag="c1")
            nc.vector.tensor_tensor(c1[:, :], rps[0:1, 1:2], b1[:, :],
                                    Alu.subtract)
            d1 = pool.tile([1, 1], dt, tag="d1")
            nc.vector.tensor_scalar(d1[:, :], c1[:, :], float(1.0 / (n - 1)),
                                    None, Alu.mult)
            e1 = pool.tile([1, 1], dt, tag="e1")
            nc.scalar.activation(e1[:, :], d1[:, :], Act.Sqrt)

            nc.sync.dma_start(o_c, CT[:, 0:L])
            nc.sync.dma_start(o_f, F[:, :])
            nc.sync.dma_start(o_olc, OLC[:, :])
            nc.sync.dma_start(o_ol, OL[:, :])
            nc.sync.dma_start(o_h, Hn[:, :])
            nc.sync.dma_start(o_l, Ln[:, :])
            nc.sync.dma_start(o_lc, LCn[:, :])
            nc.sync.dma_start(o_std, e1[:, :])
    nc.finalize()
    return nc


def _get_runner():
    global _RUNNER
    if _RUNNER is None:
        import jax
        from concourse.bass_utils import run_bass_kernel_spmd  # noqa: F401
        nc = _build()
        _RUNNER = _make_runner(nc)
    return _RUNNER


def _make_runner(nc):
    """jit-once runner (mirrors bass2jax.run_bass_via_pjrt multi-core path)."""
    import jax
    import numpy as _np
    from jax.sharding import Mesh, PartitionSpec
    from jax.experimental.shard_map import shard_map
    from concourse import mybir
    from concourse.bass2jax import (_bass_exec_p, install_neuronx_cc_hook,
                                    partition_id_tensor)

    install_neuronx_cc_hook()
    partition_name = (nc.partition_id_tensor.name
                      if nc.partition_id_tensor else None)
    in_names, out_names, out_avals, zero_outs = [], [], [], []
    for alloc in nc.m.functions[0].allocations:
        if not isinstance(alloc, mybir.MemoryLocationSet):
            continue
        name = alloc.memorylocations[0].name
        if alloc.kind == "ExternalInput":
            if name != partition_name:
                in_names.append(name)
        elif alloc.kind == "ExternalOutput":
            out_names.append(name)
            shape = tuple(alloc.tensor_shape)
            dtype = mybir.dt.np(alloc.dtype)
            out_avals.append(jax.core.ShapedArray(shape, dtype))
            zero_outs.append(_np.zeros(shape, dtype))
    n_params = len(in_names)
    all_in = in_names + out_names
    if partition_name is not None:
        all_in = all_in + [partition_name]

    def _body(*args):
        operands = list(args)
        if partition_name is not None:
            operands.append(partition_id_tensor())
        outs = _bass_exec_p.bind(
            *operands, out_avals=tuple(out_avals), in_names=tuple(all_in),
            out_names=tuple(out_names), lowering_input_output_aliases=(),
            sim_require_finite=True, sim_require_nnan=True, nc=nc)
        return tuple(outs)

    devices = jax.devices()[:NCORES]
    mesh = Mesh(_np.asarray(devices), ("core",))
    n_outs = len(out_names)
    fn = jax.jit(
        shard_map(_body, mesh=mesh,
                  in_specs=(PartitionSpec("core"),) * (n_params + n_outs),
                  out_specs=(PartitionSpec("core"),) * n_outs,
                  check_rep=False),
        keep_unused=True)

    class R:
        input_names = in_names
        output_names = out_names

        @staticmethod
        def exec(in_maps):
            per_core = [[_np.asarray(m[nm]) for nm in in_names]
                        for m in in_maps]
            cat = [_np.concatenate([per_core[c][i] for c in range(NCORES)], 0)
                   for i in range(n_params)]
            catz = [_np.concatenate([z] * NCORES, 0) for z in zero_outs]
            outs = [_np.asarray(o) for o in fn(*cat, *catz)]
            res = []
            for c in range(NCORES):
                d = {}
                for i, nm in enumerate(out_names):
                    per = outs[i].shape[0] // NCORES
                    d[nm] = outs[i][c * per:(c + 1) * per]
                res.append(d)
            return res

        @staticmethod
        def exec_async(in_maps):
            per_core = [[_np.asarray(m[nm]) for nm in in_names]
                        for m in in_maps]
            cat = [_np.concatenate([per_core[c][i] for c in range(NCORES)], 0)
                   for i in range(n_params)]
            catz = [_np.concatenate([z] * NCORES, 0) for z in zero_outs]
            return fn(*cat, *catz)

    return R


def _stage_inputs(x, y_obs, weights_vec, time_lag):
    u1 = np.asarray(x)[:, 0, 0].astype(np.float32)
    u2 = np.asarray(x)[:, 0, 1].astype(np.float32)
    tl = int(time_lag)
    u1a = np.maximum(u1, f32(TINY))
    u1a[:tl] = f32(TINY)

    pad = 128 * L
    u1g = np.full(T + 2 * pad, f32(TINY), np.float32)
    u2g = np.zeros(T + 2 * pad, np.float32)
    off = L
    u1g[off:off + T] = u1a
    u2g[off:off + T] = u2

    Wt = np.zeros((128, 8), np.float32)
    Wt[:, :5] = weights_vec  # broadcast row
    yseg = np.asarray(y_obs)[SPIN_LEN:TRAIN_LEN, 0].astype(np.float32)
    yo = np.zeros(128 * YO_COLS, np.float32)
    yo[:yseg.size] = yseg
    YOt = yo.reshape(128, YO_COLS)
    PERMt = np.zeros((128, 128), np.float32)
    for k in range(127):
        PERMt[k, k + 1] = 1.0

    in_maps = []
    for k in range(NCORES):
        g0 = k * B - L + off
        in_maps.append({
            "u1": np.ascontiguousarray(u1g[g0:g0 + 128 * L].reshape(128, L)),
            "u2": np.ascontiguousarray(u2g[g0:g0 + 128 * L].reshape(128, L)),
            "w": Wt, "yo": YOt, "perm": PERMt,
        })
    return in_maps


def kernel(x, y_obs, weight_r_yom, weight_r_ylm, weight_r_yfm,
           bias_b0_ylm, weight_b2_ylm, theltaC, epoch, time_lag):
    tl = int(time_lag)
    weights_vec = np.array([
        np.asarray(weight_r_yom).reshape(-1)[0],
        np.asarray(weight_r_ylm).reshape(-1)[0],
        np.asarray(weight_r_yfm).reshape(-1)[0],
        np.asarray(bias_b0_ylm).reshape(-1)[0],
        np.asarray(weight_b2_ylm).reshape(-1)[0],
    ], np.float32)

    in_maps = _stage_inputs(x, y_obs, weights_vec, tl)
    runner = _get_runner()
    res = runner.exec(in_maps)

    # host gather: [NCORES][128, L] -> [T] from partitions 1..NREAL
    def gather(name):
        parts = [res[k][name][1:1 + NREAL, :].reshape(-1) for k in range(NCORES)]
        return np.concatenate(parts)[:T]

    c_n = gather("o_c"); g_f = gather("o_f"); g_olc = gather("o_olc")
    g_ol = gather("o_ol"); h_n = gather("o_h"); l_n = gather("o_l")
    lc_n = gather("o_lc")
    obsstd = f32(res[0]["o_std"][0, 0])

    # oo for host fixes (host-side copy of the tiny weight math)
    eo = np.exp(weights_vec[0], dtype=np.float32)
    el = np.exp(weights_vec[1], dtype=np.float32)
    ef = np.exp(weights_vec[2], dtype=np.float32)
    oo = f32(eo / f32(eo + el + ef))
    one = f32(1.0)

    mask_inactive = np.arange(T) < tl
    czero = c_n <= f32(1e-20)
    g_f = np.where(czero, np.maximum(one - oo - g_ol, f32(0.0)), g_f)
    g_olc = np.where(czero, g_ol, g_olc)
    c_n = np.where(czero, f32(0.0), c_n)
    h_n = np.where(czero, f32(0.0), h_n)
    l_n = np.where(czero, f32(0.0), l_n)
    lc_n = np.where(czero, f32(0.0), lc_n)

    def col(a):
        return np.where(mask_inactive, f32(0.0), a).astype(np.float32)[:, None]

    h_n = col(h_n); c_n = col(c_n); l_n = col(l_n); lc_n = col(lc_n)
    g_f = col(g_f); g_olc = col(g_olc); g_ol = col(g_ol)
    g_oo = col(np.full(T, oo, np.float32))
    zeros = np.zeros((T, 1), np.float32)
    obs_col = col(np.full(T, obsstd, np.float32))
    h_nout = np.concatenate([h_n, obs_col], axis=1)

    return (h_n, c_n, l_n, lc_n, zeros, zeros.copy(), g_oo, g_ol, g_olc, g_f,
            h_nout, obs_col)
